# revision 1
# baseline (speedup 1.0000x reference)
"""Expert-parallel MoE (Mixtral-style top-2 of 8 experts, SwiGLU) on 8 TRN2 cores.

Strategy: one expert per NeuronCore. Routing (softmax/top-k/renorm) is tiny
(1024x8) and runs on host during input sharding; each core runs a dense
SwiGLU MLP over only the tokens routed to its expert (~256 of 1024, padded to
a common capacity C), with the renormalized routing weight folded in on
device. Host scatter-adds the per-expert outputs back to [T, H].

Device kernel per core (all matmuls bf16, fp32 PSUM accumulation):
  phase B: G[i, t]   = W13T[h, i].T-accum over h of x[t, h]   (gate+up halves)
           act[i, t] = silu(G_gate) * G_up                    (bf16 in SBUF)
  phase C: y[t, hh]  = sum_i act[i, t].T @ W2T[i, hh], scaled by routing w[t]

Weights are pre-transposed/tiled/bf16-cast on host so every device DMA is
fully contiguous per partition.
"""

import os

import ml_dtypes
import numpy as np

import concourse.bass as bass
from concourse import bacc
import concourse.mybir as mybir
import concourse.tile as tile
from concourse.bass_utils import run_bass_kernel_spmd

P = 128
H = 2048          # hidden dim
INTER = 4096      # intermediate dim
E = 8             # experts == cores
N_CORES = 8
HCHUNK = 512      # output column chunk (one PSUM bank of fp32)
BF16 = mybir.dt.bfloat16
F32 = mybir.dt.float32

KO = H // P           # 16 contraction steps over hidden dim
NJ = INTER // P       # 32 tiles over intermediate dim
HC = H // HCHUNK      # 4 output column chunks
NC1 = 4               # down-proj column tiles interleaved into phase B

# set by kernel() for test harness introspection
last_results = None


def _build_nc(C: int) -> bass.Bass:
    act_fn = mybir.ActivationFunctionType

    nc = bacc.Bacc()
    xt_d = nc.declare_dram_parameter("xt", [P, KO, C], BF16, isOutput=False)
    w13_d = nc.declare_dram_parameter("w13", [2 * NJ, P, KO, P], BF16, isOutput=False)
    w2_d = nc.declare_dram_parameter("w2", [NJ, P, H], BF16, isOutput=False)
    y_d = nc.declare_dram_parameter("y", [H, C], F32, isOutput=True)

    with tile.TileContext(nc) as tc:
        with (
            tc.tile_pool(name="xp", bufs=1) as xp,
            tc.tile_pool(name="w13p", bufs=4) as w13p,
            tc.tile_pool(name="w2p", bufs=1) as w2p,
            tc.tile_pool(name="actp", bufs=1) as actp,
            tc.tile_pool(name="silup", bufs=2) as silup,
            tc.tile_pool(name="outp", bufs=3) as outp,
            tc.tile_pool(name="ps13", bufs=2, space="PSUM") as ps13,
            tc.tile_pool(name="psy", bufs=2, space="PSUM") as psy,
        ):
            # first gate-weight tile loads ahead of everything so PE can
            # start ~9us in; xt slices are independent tiles so their DMAs
            # fan out across queues instead of serializing on a WAW chain
            w13_first = w13p.tile([P, KO, P], BF16, tag="w13")
            nc.sync.dma_start(w13_first[:], w13_d[0])
            xt_tiles = []
            for ko in range(KO):
                xt_sb = xp.tile([P, C], BF16, tag=f"xt_{ko}")
                nc.sync.dma_start(xt_sb[:], xt_d[:, ko, :])
                xt_tiles.append(xt_sb)

            # phase B: gate/up projections + SwiGLU, one 128-wide i-tile at a
            # time; w2 tiles are prefetched on spare DMA bandwidth as we go
            w2a_tiles, w2b_tiles = [], []
            act_tiles = []
            c1_ps = []
            for i in range(NC1):
                c1 = psy.tile([P, C], F32, tag="y", name=f"c1_{i}")
                c1_ps.append(c1)
            for j in range(NJ):
                ps_pair = []
                for k in (j, NJ + j):  # gate half, up half of W13
                    if k == 0:
                        w13_sb = w13_first
                    else:
                        w13_sb = w13p.tile([P, KO, P], BF16, tag="w13")
                        nc.sync.dma_start(w13_sb[:], w13_d[k])
                    ps = ps13.tile([P, C], F32, tag="g" if k == j else "u")
                    for ko in range(KO):
                        nc.tensor.matmul(
                            ps[:],
                            w13_sb[:, ko, :],
                            xt_tiles[ko][:],
                            start=(ko == 0),
                            stop=(ko == KO - 1),
                        )
                    ps_pair.append(ps)
                g_ps, u_ps = ps_pair

                # w2 split: the NC1 interleaved column tiles need only the
                # first NC1*P columns during phase B; the rest (w2b) loads
                # after phase B's weight stream, in time for phase C2
                w2a_sb = w2p.tile([P, NC1 * P], BF16, tag=f"w2a_{j}")
                nc.sync.dma_start(w2a_sb[:], w2_d[j][:, :NC1 * P])
                w2a_tiles.append(w2a_sb)

                # silu(g)*u as sigmoid + 2 muls (CoreSim has no Silu)
                s_sb = silup.tile([P, C], F32, tag="silu")
                nc.scalar.activation(s_sb[:], g_ps[:], act_fn.Sigmoid)
                su_sb = silup.tile([P, C], F32, tag="su")
                nc.vector.tensor_mul(su_sb[:], s_sb[:], u_ps[:])
                a_sb = actp.tile([P, C], BF16, tag=f"act_{j}")
                nc.vector.tensor_mul(a_sb[:], su_sb[:], g_ps[:])
                act_tiles.append(a_sb)

                # interleave the first NC1 down-proj column tiles into phase
                # B (lagging one j so ACT/DVE have time to produce act) --
                # keeps PE fed while the weight DMA stream catches up
                if j >= 1:
                    for hh in range(NC1):
                        nc.tensor.matmul(
                            c1_ps[hh][:],
                            w2a_tiles[j - 1][:, hh * P:(hh + 1) * P],
                            act_tiles[j - 1][:],
                            start=(j - 1 == 0),
                            stop=False,
                        )

            for j in range(NJ):
                w2b_sb = w2p.tile([P, H - NC1 * P], BF16, tag=f"w2b_{j}")
                nc.sync.dma_start(w2b_sb[:], w2_d[j][:, NC1 * P:])
                w2b_tiles.append(w2b_sb)

            def emit_down(y_ps, hh, j0, j1, start, stop):
                for j in range(j0, j1):
                    if hh < NC1:
                        w2_slice = w2a_tiles[j][:, hh * P:(hh + 1) * P]
                    else:
                        w2_slice = w2b_tiles[j][:, (hh - NC1) * P:(hh - NC1 + 1) * P]
                    nc.tensor.matmul(
                        y_ps[:],
                        w2_slice,
                        act_tiles[j][:],
                        start=start and (j == j0),
                        stop=stop and (j == j1 - 1),
                    )

            def writeback(y_ps, hh):
                o_sb = outp.tile([P, C], F32, tag="o")
                nc.vector.tensor_copy(o_sb[:], y_ps[:])
                nc.sync.dma_start(y_d[hh * P:(hh + 1) * P, :], o_sb[:])

            # finish the interleaved accumulators (last j), then drain
            for hh in range(NC1):
                emit_down(c1_ps[hh], hh, NJ - 1, NJ, start=False, stop=True)
                writeback(c1_ps[hh], hh)

            # phase C2: remaining down-proj column tiles
            for hh in range(NC1, H // P):
                y_ps = psy.tile([P, C], F32, tag="y")
                emit_down(y_ps, hh, 0, NJ, start=True, stop=True)
                writeback(y_ps, hh)
    nc.compile()
    return nc


def _route(router_logits: np.ndarray, top_k: int):
    """Match jax.nn.softmax + jax.lax.top_k + renormalize (ties -> lower idx)."""
    p = router_logits.astype(np.float64)
    p = np.exp(p - p.max(axis=-1, keepdims=True))
    p /= p.sum(axis=-1, keepdims=True)
    order = np.argsort(-p, axis=-1, kind="stable")
    idx = order[:, :top_k]
    w = np.take_along_axis(p, idx, axis=-1)
    w /= w.sum(axis=-1, keepdims=True)
    return idx, w


def kernel(hidden_states, router_logits, W13, W2, top_k):
    global last_results
    top_k = int(top_k)
    hs = np.asarray(hidden_states, dtype=np.float32)
    T = hs.shape[0]
    idx, w = _route(np.asarray(router_logits, dtype=np.float32), top_k)

    tok_ids, tok_w = [], []
    for e in range(E):
        sel = idx == e  # [T, k]; at most one True per row
        rows = np.nonzero(sel.any(axis=-1))[0]
        tok_ids.append(rows)
        tok_w.append(w[sel].astype(np.float32))  # row-major -> token order

    C = max(16, -(-max(len(r) for r in tok_ids) // 16) * 16)

    W13 = np.asarray(W13, dtype=np.float32)
    W2 = np.asarray(W2, dtype=np.float32)
    in_maps = []
    for e in range(E):
        rows = tok_ids[e]
        n_e = len(rows)
        xt = np.zeros((P, KO, C), dtype=ml_dtypes.bfloat16)
        if n_e:
            xg = hs[rows].astype(ml_dtypes.bfloat16)  # [n_e, H]
            xt[:, :, :n_e] = xg.T.reshape(KO, P, n_e).transpose(1, 0, 2)
        w13 = np.ascontiguousarray(
            W13[e].astype(ml_dtypes.bfloat16)
            .reshape(2 * NJ, P, KO, P).transpose(0, 3, 2, 1)
        )
        w2 = np.ascontiguousarray(
            W2[e].astype(ml_dtypes.bfloat16)
            .reshape(H, NJ, P).transpose(1, 2, 0)
        )
        in_maps.append({"xt": xt, "w13": w13, "w2": w2})

    nc = _build_nc(C)
    res = run_bass_kernel_spmd(
        nc,
        in_maps,
        list(range(N_CORES)),
        trace=bool(os.environ.get("MOE_TRACE")),
        tmpdir=os.environ.get("MOE_TRACE_DIR") or None,
    )
    last_results = res

    out = np.zeros((T, H), dtype=np.float32)
    for e in range(E):
        rows = tok_ids[e]
        n_e = len(rows)
        if n_e:
            y = res.results[e]["y"]  # [H, C]
            out[rows] += y[:, :n_e].T * tok_w[e][:, None]
    return out



# revision 2
# speedup vs baseline: 1.1012x; 1.1012x over previous
"""Expert-parallel MoE (Mixtral-style top-2 of 8 experts, SwiGLU) on 8 TRN2 cores.

Two-slot layout: every core hosts HALF (along the intermediate dim) of two
experts — one "heavy" expert (slot 0, column capacity C_a) and one "light"
expert (slot 1, C_b).  Each expert's I=4096 channels are split across two
cores; the two half-results are summed on host.  This balances PE work across
cores: streaming cycles drop from 1536*C_max to 768*(C_a+C_b).

Routing (softmax/top-k/renorm, 1024x8) runs on host during input sharding;
the routing weight is folded in on host during the combine.

Device kernel per core, per slot (all matmuls bf16, fp32 PSUM accumulation):
  phase B:  G[i,t] = W13T[h,i].T-accum over h of x[h,t]  (gate|up pairs)
            act[i,t] = sigmoid(G_g)*G_g*G_u              (bf16 in SBUF)
  phase D:  y[hh,t] = sum_i act[i,t].T @ W2T[i,hh]       (j-major waves of
            <=6 PSUM accumulators so W2 tiles are consumed in DMA order)

DMA notes: every DMA instruction costs ~0.6us of sequencer issue time, so
weights stream on the Sync ring while xt/y ride the Scalar (ACT) ring, and
the first tiles are split into small chunks so the PE can start ~8us in.
"""

import os

import ml_dtypes
import numpy as np

import concourse.bass as bass
from concourse import bacc
import concourse.mybir as mybir
import concourse.tile as tile
from concourse.bass_utils import run_bass_kernel_spmd

P = 128
H = 2048            # hidden dim
INTER = 4096        # intermediate dim
E = 8               # experts
N_CORES = 8
N_SLOTS = 2         # experts hosted per core (half-I each)
IH = INTER // N_SLOTS   # 2048 intermediate channels per slot
KO = H // P         # 16 contraction steps over hidden dim
NJ = IH // P        # 16 i-tiles per slot
HC = H // P         # 16 output row tiles
WAVES = [(0, 6), (6, 6), (12, 4)]   # down-proj hh waves (PSUM: 6+2 banks max)
# first-load chunking (ko ranges) for slot 0 so the PE can start early
HEAD_CHUNKS = [(0, 1), (1, 3), (4, 4), (8, 8)]
XB_CHUNKS = [(0, 4), (4, 4), (8, 4), (12, 4)]
BF16 = mybir.dt.bfloat16
F32 = mybir.dt.float32

# set by kernel() for test harness introspection
last_results = None


def _build_nc(caps: list[int]) -> bass.Bass:
    act_fn = mybir.ActivationFunctionType
    CMAX = max(caps)

    nc = bacc.Bacc()
    xt_d, w13_d, w2_d, y_d = [], [], [], []
    for s, C in enumerate(caps):
        xt_d.append(nc.declare_dram_parameter(f"xt{s}", [P, KO, C], BF16, isOutput=False))
        w13_d.append(nc.declare_dram_parameter(f"w13{s}", [NJ, P, KO, 2 * P], BF16, isOutput=False))
        w2_d.append(nc.declare_dram_parameter(f"w2{s}", [NJ, P, H], BF16, isOutput=False))
        y_d.append(nc.declare_dram_parameter(f"y{s}", [H, C], BF16, isOutput=True))

    with tile.TileContext(nc) as tc:
        with (
            tc.tile_pool(name="xp", bufs=1) as xp,
            tc.tile_pool(name="w13p", bufs=8) as w13p,
            tc.tile_pool(name="w13hp", bufs=1) as w13hp,
            tc.tile_pool(name="w2p", bufs=1) as w2p,
            tc.tile_pool(name="actp", bufs=1) as actp,
            tc.tile_pool(name="silup", bufs=2) as silup,
            tc.tile_pool(name="outp", bufs=3) as outp,
            tc.tile_pool(name="psB", bufs=1, space="PSUM") as psB,
            tc.tile_pool(name="psD", bufs=1, space="PSUM") as psD,
        ):
            # slot-0 xt in ramped chunks on the ACT ring; weights own the SP ring
            def load_xt(s, chunks):
                tiles = [None] * KO
                for ci, (k0, kn) in enumerate(chunks):
                    t = xp.tile([P, kn, CMAX], BF16, tag=f"xc_{s}_{ci}", name=f"xc{s}_{ci}")
                    nc.scalar.dma_start(t[:, :, : caps[s]], xt_d[s][:, k0 : k0 + kn, :])
                    for r in range(kn):
                        tiles[k0 + r] = t[:, r, :]
                return tiles

            xt_all = [load_xt(0, HEAD_CHUNKS)] + [None] * (N_SLOTS - 1)

            # slot-0 j=0 weights in the same ramped chunks (SP ring)
            heads = []
            for ci, (k0, kn) in enumerate(HEAD_CHUNKS):
                ht = w13hp.tile([P, kn, 2 * P], BF16, tag=f"h_{ci}", name=f"w13h{ci}")
                nc.sync.dma_start(ht[:], w13_d[0][0][:, k0 : k0 + kn, :])
                heads.append(ht)

            def head_slice(ko, half):
                for ci, (k0, kn) in enumerate(HEAD_CHUNKS):
                    if k0 <= ko < k0 + kn:
                        return heads[ci][:, ko - k0, half * P : (half + 1) * P]
                raise AssertionError

            for s, C in enumerate(caps):
                xt_tiles = xt_all[s]
                act_tiles = []
                w2_tiles = []
                # ---- phase B: gate/up projections + SwiGLU ----
                for j in range(NJ):
                    if s == 0 and j == 0:
                        w13_sl = head_slice
                    else:
                        w13_sb = w13p.tile([P, KO, 2 * P], BF16, tag="w13", name=f"w13_{s}_{j}")
                        nc.sync.dma_start(w13_sb[:], w13_d[s][j])

                        def w13_sl(ko, half, _t=w13_sb):
                            return _t[:, ko, half * P : (half + 1) * P]

                    g_ps = psB.tile([P, CMAX], F32, tag="g", name=f"g_{s}_{j}")
                    u_ps = psB.tile([P, CMAX], F32, tag="u", name=f"u_{s}_{j}")
                    for half, ps in ((0, g_ps), (1, u_ps)):
                        for ko in range(KO):
                            nc.tensor.matmul(
                                ps[:, :C],
                                w13_sl(ko, half),
                                xt_tiles[ko][:, :C],
                                start=(ko == 0),
                                stop=(ko == KO - 1),
                            )
                    # prefetch next slot's xt on the ACT ring, spread over phase B
                    if s + 1 < N_SLOTS and j < len(XB_CHUNKS):
                        if xt_all[s + 1] is None:
                            xt_all[s + 1] = [None] * KO
                        k0, kn = XB_CHUNKS[j]
                        t = xp.tile([P, kn, CMAX], BF16, tag=f"xc_{s+1}_{j}", name=f"xc{s+1}_{j}")
                        nc.scalar.dma_start(t[:, :, : caps[s + 1]], xt_d[s + 1][:, k0 : k0 + kn, :])
                        for r in range(kn):
                            xt_all[s + 1][k0 + r] = t[:, r, :]
                    # silu: s*g first so the g PSUM bank frees before gate j+1
                    s_sb = silup.tile([P, CMAX], F32, tag="sig", name=f"sig_{s}_{j}")
                    nc.scalar.activation(s_sb[:, :C], g_ps[:, :C], act_fn.Sigmoid)
                    sg_sb = silup.tile([P, CMAX], F32, tag="sg", name=f"sg_{s}_{j}")
                    nc.vector.tensor_mul(sg_sb[:, :C], s_sb[:, :C], g_ps[:, :C])
                    a_sb = actp.tile([P, CMAX], BF16, tag=f"act_{j}", name=f"act_{s}_{j}")
                    nc.vector.tensor_mul(a_sb[:, :C], sg_sb[:, :C], u_ps[:, :C])
                    act_tiles.append(a_sb)

                # down-proj weights queue on the SP ring behind this slot's w13s
                for j in range(NJ):
                    w2_sb = w2p.tile([P, H], BF16, tag=f"w2_{j}", name=f"w2_{s}_{j}")
                    nc.sync.dma_start(w2_sb[:], w2_d[s][j])
                    w2_tiles.append(w2_sb)

                # ---- phase D: down-proj, j-major waves over hh ----
                for wstart, wn in WAVES:
                    accs = [
                        psD.tile([P, CMAX], F32, tag=f"acc_{k}", name=f"acc_{s}_{wstart}_{k}")
                        for k in range(wn)
                    ]
                    for j in range(NJ):
                        for k in range(wn):
                            hh = wstart + k
                            nc.tensor.matmul(
                                accs[k][:, :C],
                                w2_tiles[j][:, hh * P : (hh + 1) * P],
                                act_tiles[j][:, :C],
                                start=(j == 0),
                                stop=(j == NJ - 1),
                            )
                            if j == NJ - 1:
                                # copy each acc right after its stop-MM so the
                                # next wave's first MMs aren't blocked
                                o_sb = outp.tile([P, CMAX], BF16, tag="o", name=f"o_{s}_{hh}")
                                nc.vector.tensor_copy(o_sb[:, :C], accs[k][:, :C])
                                nc.scalar.dma_start(y_d[s][hh * P : (hh + 1) * P, :], o_sb[:, :C])
    nc.compile()
    return nc


def _route(router_logits: np.ndarray, top_k: int):
    """Match jax.nn.softmax + jax.lax.top_k + renormalize (ties -> lower idx)."""
    p = router_logits.astype(np.float64)
    p = np.exp(p - p.max(axis=-1, keepdims=True))
    p /= p.sum(axis=-1, keepdims=True)
    order = np.argsort(-p, axis=-1, kind="stable")
    idx = order[:, :top_k]
    w = np.take_along_axis(p, idx, axis=-1)
    w /= w.sum(axis=-1, keepdims=True)
    return idx, w


def kernel(hidden_states, router_logits, W13, W2, top_k):
    global last_results
    top_k = int(top_k)
    hs = np.asarray(hidden_states, dtype=np.float32)
    T = hs.shape[0]
    idx, w = _route(np.asarray(router_logits, dtype=np.float32), top_k)

    tok_ids, tok_w = [], []
    for e in range(E):
        sel = idx == e  # [T, k]; at most one True per row
        rows = np.nonzero(sel.any(axis=-1))[0]
        tok_ids.append(rows)
        tok_w.append(w[sel].astype(np.float32))  # row-major -> token order

    counts = np.array([len(r) for r in tok_ids])
    order = np.argsort(-counts, kind="stable")
    groups = [order[:4], order[4:]]  # heavy experts in slot 0, light in slot 1

    def pad16(n):
        return max(16, -(-n // 16) * 16)

    caps = [pad16(int(counts[g].max())) for g in groups]
    assert caps[0] <= 512, "column capacity exceeds one PSUM bank"

    W13 = np.asarray(W13, dtype=np.float32)
    W2 = np.asarray(W2, dtype=np.float32)

    in_maps = [dict() for _ in range(N_CORES)]
    for c in range(N_CORES):
        for s in range(N_SLOTS):
            e = int(groups[s][c // 2])
            h = c % 2  # which half of the expert's I channels
            C = caps[s]
            rows = tok_ids[e]
            n = len(rows)
            xt = np.zeros((P, KO, C), dtype=ml_dtypes.bfloat16)
            if n:
                xg = hs[rows].astype(ml_dtypes.bfloat16)  # [n, H]
                xt[:, :, :n] = xg.T.reshape(KO, P, n).transpose(1, 0, 2)
            gate = W13[e][h * IH : (h + 1) * IH]                    # [IH, H]
            up = W13[e][INTER + h * IH : INTER + (h + 1) * IH]     # [IH, H]
            blk = np.concatenate(
                [gate.reshape(NJ, P, H), up.reshape(NJ, P, H)], axis=1
            )  # [NJ, 2P(i), H]
            w13 = np.ascontiguousarray(
                blk.reshape(NJ, 2 * P, KO, P).transpose(0, 3, 2, 1)
            ).astype(ml_dtypes.bfloat16)  # [NJ, P(h), KO, 2P(i)]
            cols = W2[e][:, h * IH : (h + 1) * IH]  # [H, IH]
            w2 = np.ascontiguousarray(cols.T.reshape(NJ, P, H)).astype(
                ml_dtypes.bfloat16
            )  # [NJ, P(i), H]
            in_maps[c][f"xt{s}"] = xt
            in_maps[c][f"w13{s}"] = w13
            in_maps[c][f"w2{s}"] = w2

    nc = _build_nc(caps)
    res = run_bass_kernel_spmd(
        nc,
        in_maps,
        list(range(N_CORES)),
        trace=bool(os.environ.get("MOE_TRACE")),
        tmpdir=os.environ.get("MOE_TRACE_DIR") or None,
    )
    last_results = res

    out = np.zeros((T, H), dtype=np.float32)
    for s in range(N_SLOTS):
        for k in range(len(groups[s])):
            e = int(groups[s][k])
            rows = tok_ids[e]
            n = len(rows)
            if not n:
                continue
            y0 = np.asarray(res.results[2 * k][f"y{s}"], dtype=np.float32)
            y1 = np.asarray(res.results[2 * k + 1][f"y{s}"], dtype=np.float32)
            out[rows] += (y0[:, :n] + y1[:, :n]).T * tok_w[e][:, None]
    return out
